# revision 16
# baseline (speedup 1.0000x reference)
"""GCN 2-layer kernel for Trainium2, 8 NeuronCores, single SPMD launch.

out = log_softmax(Ahat @ relu(Ahat @ (x@W1) + b1) @ W2 + b2),
Ahat = D^-1/2 (A+I) D^-1/2.

Rewritten (dinv scaling folded into per-node pre/post scales):
  g1 = dinv * (x @ W1)            [N,16]   bf16 matmul, per-core rows
  s1 = sum_{e: dst=v} g1[src_e]            ELL gather + reduce
  g2 = dinv * relu(dinv * s1 + b1)
  s2 = sum g2[src_e]
  out = log_softmax((dinv * s2) @ W2 + b2)

Single Bass program per core, one launch:
  phase A  : x slab (host-transposed bf16) -> matmul W1 -> g1 rows (For_i)
  AllGather: g1 [RT,16] -> tab1 [8*RT,16] (internal shared DRAM)
  layer 1  : ELL gather decomposed into "rectangles" (tile-range x fixed
             column count); each rectangle is one For_i hardware loop whose
             body issues the per-column indirect DMAs + partial reduce into
             a per-tile accumulator. Keeps the static instruction count
             ~100/layer while the ~3300 dynamic gathers run ~1.45us each.
  AllGather: g2 -> tab2
  layer 2  : same gather, then W2 matmul + log_softmax (For_i) -> out

Host prep: graph partition by dst across cores, degree-sorted ELL layout,
dinv scales, per-rectangle index tables (natural order for layer 1,
degree-sorted order for layer 2). Output rows un-permuted on host.
"""
import sys
sys.path.insert(0, "/opt/trn_rl_repo")
import numpy as np
import ml_dtypes

import concourse.bass as bass
from concourse.bass import ds, ts
import concourse.bacc as bacc
import concourse.mybir as mybir
import concourse.tile as tile
import concourse.bass_utils as bass_utils
from concourse.masks import make_identity

F32 = mybir.dt.float32
BF16 = mybir.dt.bfloat16
FP8 = mybir.dt.float8e4
I32 = mybir.dt.int32

M_CORES = 8


def _rectangles(KS, max_rects=8):
    """Cover the (descending) ELL column staircase with rectangles.

    Returns [(n_tiles, c_lo, c_hi)]: rectangle = tiles [0, n_tiles) x
    columns [c_lo, c_hi). Greedy: split at the largest staircase drops.
    """
    KS = list(KS)
    NT = len(KS)
    assert all(KS[i] >= KS[i + 1] for i in range(NT - 1)), "KS must descend"
    # candidate breakpoints: distinct K values (descending staircase)
    # choose levels greedily by waste reduction
    levels = sorted(set(KS))           # ascending
    base = levels[0]
    chosen = {0, base}
    # waste reduction of adding level c between existing neighbours:
    # evaluate greedily
    def total_waste(lvls):
        lv = sorted(lvls)
        waste = 0
        for t, k in enumerate(KS):
            # covered columns: for each adjacent pair (a, b] need n_tiles with
            # K >= b; per tile, covered = smallest chosen level >= k
            cov = min(l for l in lv if l >= k)
            waste += cov - k
        return waste
    chosen.add(max(KS))
    while len(chosen) < max_rects + 1:
        best, bestw = None, total_waste(chosen)
        for c in levels:
            if c in chosen:
                continue
            w = total_waste(chosen | {c})
            if w < bestw:
                best, bestw = c, w
        if best is None:
            break
        chosen.add(best)
    lv = sorted(c for c in chosen if c > 0)
    rects = []
    prev = 0
    for c in lv:
        n = sum(1 for k in KS if k > prev)       # tiles needing cols > prev
        if n == 0 or c == prev:
            prev = c
            continue
        rects.append((n, prev, c))
        prev = c
    return rects


def _build(NT, D_IN, H, C, KS, rects, n_cores=M_CORES):
    RT = NT * 128
    RTZ = RT + 16                  # 16 trailing zero rows per rank
    KD = D_IN // 128
    NTAB = n_cores * RTZ
    # per-rect index table column offsets
    CR = [n * (hi - lo) for (n, lo, hi) in rects]
    CTOT = int(sum(CR))
    roff = np.concatenate([[0], np.cumsum(CR)]).astype(int)

    nc = bacc.Bacc("TRN2", target_bir_lowering=False, debug=False,
                   num_devices=n_cores)
    xt_ap = nc.dram_tensor("xt", [KD * 128, RT], FP8, kind="ExternalInput").ap()
    w1_ap = nc.dram_tensor("w1", [128, KD * H], BF16, kind="ExternalInput").ap()
    w2_ap = nc.dram_tensor("w2", [H, C], F32, kind="ExternalInput").ap()
    b1_ap = nc.dram_tensor("b1", [128, H], F32, kind="ExternalInput").ap()
    b2_ap = nc.dram_tensor("b2", [128, C], F32, kind="ExternalInput").ap()
    dvn_ap = nc.dram_tensor("dvn", [128, NT], F32, kind="ExternalInput").ap()
    dvp_ap = nc.dram_tensor("dvp", [128, NT], F32, kind="ExternalInput").ap()
    ix1_ap = nc.dram_tensor("ix1", [128, CTOT], I32, kind="ExternalInput").ap()
    ix2_ap = nc.dram_tensor("ix2", [128, CTOT], I32, kind="ExternalInput").ap()
    out_ap = nc.dram_tensor("out", [RT, C], F32, kind="ExternalOutput").ap()

    rg = [list(range(n_cores))]

    with tile.TileContext(nc) as tc:
        with tc.tile_pool(name="dram", bufs=1, space="DRAM") as dpool, \
             tc.tile_pool(name="const", bufs=1) as cpool, \
             tc.tile_pool(name="work", bufs=4) as wpool, \
             tc.tile_pool(name="gath", bufs=4) as gpool, \
             tc.tile_pool(name="psA", bufs=2, space="PSUM") as psA, \
             tc.tile_pool(name="psT", bufs=2, space="PSUM") as psT:

            # ---- constants
            ident = cpool.tile([128, 128], F32)
            make_identity(nc, ident[:])
            w1_t = cpool.tile([128, KD * H], BF16)
            nc.sync.dma_start(out=w1_t[:], in_=w1_ap[:])
            w2_t = cpool.tile([H, C], F32)
            nc.sync.dma_start(out=w2_t[:], in_=w2_ap[:])
            b1_t = cpool.tile([128, H], F32)
            nc.sync.dma_start(out=b1_t[:], in_=b1_ap[:])
            b2_t = cpool.tile([128, C], F32)
            nc.sync.dma_start(out=b2_t[:], in_=b2_ap[:])
            dvn_t = cpool.tile([128, NT], F32)
            nc.sync.dma_start(out=dvn_t[:], in_=dvn_ap[:])
            dvp_t = cpool.tile([128, NT], F32)
            nc.sync.dma_start(out=dvp_t[:], in_=dvp_ap[:])
            ix1_t = cpool.tile([128, CTOT], I32)
            nc.sync.dma_start(out=ix1_t[:], in_=ix1_ap[:])
            ix2_t = cpool.tile([128, CTOT], I32)
            nc.sync.dma_start(out=ix2_t[:], in_=ix2_ap[:])

            # per-tile partial-sum accumulator [128, NT*H]
            s_acc = cpool.tile([128, NT * H], F32)

            # ---- DRAM intermediates (trailing 16 zero rows per rank feed
            # the ELL padding slots after the AllGather)
            g1l = dpool.tile([RTZ, H], F32)
            tab1 = dpool.tile([NTAB, H], F32, addr_space="Shared")
            g2l = dpool.tile([RTZ, H], F32)
            tab2 = dpool.tile([NTAB, H], F32, addr_space="Shared")

            zt = cpool.tile([128, H], F32)
            nc.vector.memset(zt[:], 0.0)
            nc.sync.dma_start(out=g1l[RT:, :], in_=zt[0:16, :])
            nc.sync.dma_start(out=g2l[RT:, :], in_=zt[0:16, :])

            # ---- phase A: g1 = dvn * (x @ W1), x tiles streamed as bf16
            xt_k = xt_ap.rearrange("(k p) c -> p k c", p=128)
            with tc.For_i(0, NT, 1, name="phA") as iv:
                xt_t = gpool.tile([128, KD * 128], BF16, tag="xin")
                nc.gpsimd.dma_start(
                    out=xt_t[:].rearrange("p (k c) -> p k c", k=KD),
                    in_=xt_k[:, :, ds(iv * 128, 128)])
                acc = psA.tile([128, H], F32, tag="accA")
                for k in range(KD):
                    nc.tensor.matmul(
                        out=acc[:],
                        lhsT=xt_t[:, k * 128:(k + 1) * 128],
                        rhs=w1_t[:, k * H:(k + 1) * H],
                        start=(k == 0), stop=(k == KD - 1))
                gt = wpool.tile([128, H], F32, tag="gout")
                nc.vector.tensor_scalar_mul(gt[:], acc[:], dvn_t[:, ts(iv, 1)])
                nc.sync.dma_start(out=g1l[ts(iv, 128), :], in_=gt[:])

            # ---- AllGather 1
            nc.gpsimd.collective_compute(
                "AllGather", mybir.AluOpType.bypass, replica_groups=rg,
                ins=[g1l[:].opt()], outs=[tab1[:].opt()])

            def gather_layer(ix_t, tab):
                """Rectangle loops: gather + partial reduce into s_acc."""
                for r, (n, lo, hi) in enumerate(rects):
                    dc = hi - lo
                    with tc.For_i(0, n, 1, name=f"g{r}") as iv:
                        ixs = gpool.tile([128, dc], I32, tag="ixs")
                        nc.vector.tensor_copy(
                            ixs[:], ix_t[:, ds(int(roff[r]) + iv * dc, dc)])
                        ell = gpool.tile([128, dc * H], F32, tag="ell")
                        for j in range(dc):
                            nc.gpsimd.indirect_dma_start(
                                out=ell[:, j * H:(j + 1) * H],
                                out_offset=None,
                                in_=tab[:],
                                in_offset=bass.IndirectOffsetOnAxis(
                                    ap=ixs[:, j:j + 1], axis=0),
                            )
                        if dc > 1:
                            s = wpool.tile([128, H], F32, tag="s")
                            nc.vector.reduce_sum(
                                out=s[:],
                                in_=ell[:].rearrange("p (k h) -> p h k", h=H),
                                axis=mybir.AxisListType.X)
                        else:
                            s = ell
                        if r == 0:
                            nc.any.tensor_copy(s_acc[:, ts(iv, H)], s[:])
                        else:
                            nc.vector.tensor_add(
                                s_acc[:, ts(iv, H)],
                                s_acc[:, ts(iv, H)], s[:])

            # ---- layer 1
            gather_layer(ix1_t, tab1)
            with tc.For_i(0, NT, 1, name="post1") as iv:
                a = wpool.tile([128, H], F32, tag="p1a")
                nc.vector.tensor_scalar_mul(
                    a[:], s_acc[:, ts(iv, H)], dvp_t[:, ts(iv, 1)])
                nc.vector.tensor_add(a[:], a[:], b1_t[:])
                r1 = wpool.tile([128, H], F32, tag="p1r")
                nc.scalar.activation(r1[:], a[:],
                                     mybir.ActivationFunctionType.Relu)
                nc.vector.tensor_scalar_mul(r1[:], r1[:], dvp_t[:, ts(iv, 1)])
                nc.sync.dma_start(out=g2l[ts(iv, 128), :], in_=r1[:])

            # ---- AllGather 2
            nc.gpsimd.collective_compute(
                "AllGather", mybir.AluOpType.bypass, replica_groups=rg,
                ins=[g2l[:].opt()], outs=[tab2[:].opt()])

            # ---- layer 2
            gather_layer(ix2_t, tab2)
            with tc.For_i(0, NT, 1, name="post2") as iv:
                a = wpool.tile([128, H], F32, tag="p2a")
                nc.vector.tensor_scalar_mul(
                    a[:], s_acc[:, ts(iv, H)], dvp_t[:, ts(iv, 1)])
                ptr = psT.tile([128, 128], F32, tag="ptr2")
                nc.tensor.transpose(out=ptr[:H, :], in_=a[:, :],
                                    identity=ident[:])
                aT = wpool.tile([H, 128], F32, tag="aT")
                nc.any.tensor_copy(aT[:], ptr[:H, :])
                lg = psA.tile([128, C], F32, tag="lg")
                nc.tensor.matmul(out=lg[:], lhsT=aT[:], rhs=w2_t[:],
                                 start=True, stop=True)
                z = wpool.tile([128, C], F32, tag="z")
                nc.vector.tensor_add(z[:], lg[:], b2_t[:])
                mx = wpool.tile([128, 1], F32, tag="mx")
                nc.vector.reduce_max(out=mx[:], in_=z[:],
                                     axis=mybir.AxisListType.X)
                nc.vector.tensor_scalar(
                    out=z[:], in0=z[:], scalar1=mx[:, 0:1], scalar2=None,
                    op0=mybir.AluOpType.subtract)
                e = wpool.tile([128, C], F32, tag="e")
                nc.scalar.activation(e[:], z[:],
                                     mybir.ActivationFunctionType.Exp)
                se = wpool.tile([128, 1], F32, tag="se")
                nc.vector.reduce_sum(out=se[:], in_=e[:],
                                     axis=mybir.AxisListType.X)
                ls = wpool.tile([128, 1], F32, tag="ls")
                nc.scalar.activation(ls[:], se[:],
                                     mybir.ActivationFunctionType.Ln)
                nc.vector.tensor_scalar(
                    out=z[:], in0=z[:], scalar1=ls[:, 0:1], scalar2=None,
                    op0=mybir.AluOpType.subtract)
                nc.sync.dma_start(out=out_ap[ts(iv, 128), :], in_=z[:])

    nc.compile()
    return nc


def _host_prep(x, edge_index, W1, b1, W2, b2, n_cores=M_CORES):
    x = np.asarray(x, dtype=np.float32)
    N, D_IN = x.shape
    W1 = np.asarray(W1, np.float32)
    W2 = np.asarray(W2, np.float32)
    H = W1.shape[1]
    C = W2.shape[1]
    NPC = N // n_cores
    NT = (NPC + 127) // 128
    RT = NT * 128
    RTZ = RT + 16
    ZROW = RT                     # rank 0's zero-row block
    KD = D_IN // 128

    src = np.asarray(edge_index[0], dtype=np.int32)
    dst = np.asarray(edge_index[1], dtype=np.int32)
    deg = np.bincount(dst, minlength=N).astype(np.float64) + 1.0
    dinv = (1.0 / np.sqrt(deg)).astype(np.float32)

    owner = dst // NPC
    np.minimum(owner, n_cores - 1, out=owner)

    per_core = []
    KS_all = np.zeros((n_cores, NT), dtype=np.int64)
    for m in range(n_cores):
        sel = owner == m
        s_m = src[sel]
        d_m = dst[sel] - m * NPC
        s_m = np.concatenate([s_m, np.arange(m * NPC, (m + 1) * NPC,
                                             dtype=np.int32)])
        d_m = np.concatenate([d_m, np.arange(NPC, dtype=np.int32)])
        degl = np.bincount(d_m, minlength=NPC)
        perm = np.argsort(-degl, kind="stable").astype(np.int32)
        inv_perm = np.empty(NPC, dtype=np.int32)
        inv_perm[perm] = np.arange(NPC, dtype=np.int32)
        degs = degl[perm]
        Ks = np.zeros(NT, dtype=np.int64)
        nfull = NPC // 128
        for t in range(nfull):
            Ks[t] = degs[t * 128]
        if NPC % 128:
            Ks[nfull] = degs[nfull * 128] if nfull * 128 < NPC else 0
        per_core.append(dict(s_m=s_m, d_m=d_m, perm=perm, inv_perm=inv_perm,
                             degl=degl))
        KS_all[m] = Ks
    KS = KS_all.max(axis=0)
    KS = np.maximum(KS, 1)

    rects = _rectangles([int(k) for k in KS])
    CR = [n * (hi - lo) for (n, lo, hi) in rects]
    CTOT = int(sum(CR))
    roff = np.concatenate([[0], np.cumsum(CR)]).astype(int)

    # map (tile, col) -> index-table column (per-rectangle layout)
    def ixcol(t_idx, col):
        # returns flat column in the per-rect table; t_idx, col arrays
        res = np.empty_like(t_idx)
        for r, (n, lo, hi) in enumerate(rects):
            selr = (col >= lo) & (col < hi)
            res[selr] = roff[r] + t_idx[selr] * (hi - lo) + (col[selr] - lo)
        return res

    # global node -> table row maps (built once, used per core)
    all_own = np.minimum(np.arange(N) // NPC, n_cores - 1)
    all_loc = np.arange(N) - all_own * NPC
    row_nat = all_own * RTZ + all_loc
    sorted_pos = np.empty(N, dtype=np.int32)
    for j in range(n_cores):
        sorted_pos[j * NPC:(j + 1) * NPC] = per_core[j]["inv_perm"]
    row_prm = all_own * RTZ + sorted_pos

    def table_rows(nodes, permuted):
        return (row_prm if permuted else row_nat)[nodes]

    ix1 = np.full((n_cores, 128, CTOT), ZROW, dtype=np.int32)
    ix2 = np.full((n_cores, 128, CTOT), ZROW, dtype=np.int32)
    dvn = np.ones((n_cores, 128, NT), dtype=np.float32)
    dvp = np.ones((n_cores, 128, NT), dtype=np.float32)

    for m in range(n_cores):
        pc = per_core[m]
        s_m, d_m = pc["s_m"], pc["d_m"]
        spos = pc["inv_perm"][d_m]
        order = np.argsort(spos, kind="stable")
        s_srt = s_m[order]
        p_srt = spos[order]
        counts = pc["degl"][pc["perm"]]
        offs = np.concatenate([[0], np.cumsum(counts)])
        rank = np.arange(len(p_srt)) - offs[p_srt]
        t_idx = p_srt // 128
        p_row = p_srt % 128
        colpos = ixcol(t_idx, rank)
        r1 = table_rows(s_srt, permuted=False)
        r2 = table_rows(s_srt, permuted=True)
        ix1[m, p_row, colpos] = r1
        ix2[m, p_row, colpos] = r2
        own_nodes = np.arange(m * NPC, (m + 1) * NPC)
        dv = dinv[own_nodes]
        nat = np.ones(RT, np.float32)
        nat[:NPC] = dv
        dvn[m] = nat.reshape(NT, 128).T
        prm = np.ones(RT, np.float32)
        prm[:NPC] = dv[pc["perm"]]
        dvp[m] = prm.reshape(NT, 128).T

    x_pad8 = np.zeros((N + RT, D_IN), ml_dtypes.float8_e4m3)
    x_pad8[:N] = x.astype(ml_dtypes.float8_e4m3)
    w1b = np.ascontiguousarray(
        W1.reshape(KD, 128, H).transpose(1, 0, 2).reshape(128, -1)
    ).astype(ml_dtypes.bfloat16)
    b1f = np.tile(np.asarray(b1, np.float32)[None, :], (128, 1))
    b2f = np.tile(np.asarray(b2, np.float32)[None, :], (128, 1))
    in_maps = []
    for m in range(n_cores):
        xs = x_pad8[m * NPC:m * NPC + RT]
        xt = np.ascontiguousarray(xs.T)
        in_maps.append({
            "xt": xt,
            "w1": w1b,
            "w2": W2,
            "b1": b1f,
            "b2": b2f,
            "dvn": dvn[m], "dvp": dvp[m],
            "ix1": ix1[m], "ix2": ix2[m],
        })
    meta = dict(NPC=NPC, NT=NT, RT=RT, KS=[int(k) for k in KS],
                rects=rects, perms=[pc["perm"] for pc in per_core])
    return in_maps, meta


_CACHE = {}


def kernel(x, edge_index, W1, b1, W2, b2):
    x = np.asarray(x)
    n_cores = M_CORES
    N, D_IN = x.shape
    H = np.asarray(W1).shape[1]
    C = np.asarray(W2).shape[1]
    in_maps, meta = _host_prep(x, edge_index, W1, b1, W2, b2, n_cores)
    NPC, NT = meta["NPC"], meta["NT"]
    key = (N, D_IN, H, C, tuple(meta["KS"]))
    if key not in _CACHE:
        _CACHE[key] = _build(NT, D_IN, H, C, meta["KS"], meta["rects"],
                             n_cores)
    nc = _CACHE[key]
    res = bass_utils.run_bass_kernel_spmd(nc, in_maps,
                                          core_ids=list(range(n_cores)))
    out = np.empty((N, C), np.float32)
    for m in range(n_cores):
        om = res.results[m]["out"]
        out[m * NPC + meta["perms"][m]] = om[:NPC]
    return out


# revision 18
# speedup vs baseline: 1.3218x; 1.3218x over previous
"""GCN 2-layer kernel for Trainium2, 8 NeuronCores, single SPMD launch.

out = log_softmax(Ahat @ relu(Ahat @ (x@W1) + b1) @ W2 + b2),
Ahat = D^-1/2 (A+I) D^-1/2.

Rewritten (dinv scaling folded into per-node pre/post scales):
  g1 = dinv * (x @ W1)            [N,16]   bf16 matmul, per-core rows
  s1 = sum_{e: dst=v} g1[src_e]            ELL gather + reduce
  g2 = dinv * relu(dinv * s1 + b1)
  s2 = sum g2[src_e]
  out = log_softmax((dinv * s2) @ W2 + b2)

Single Bass program per core, one launch:
  phase A  : x uploaded fp8 (halves the dominant transfer), cast to bf16
             during the SWDGE load DMA, bf16 matmul W1 -> g1 rows (For_i)
  AllGather: g1 [RT,16] -> tab1 [8*RT,16] (internal shared DRAM)
  layer 1  : ELL gather decomposed into "rectangles" (tile-range x fixed
             column count); each rectangle is one For_i hardware loop whose
             body issues the per-column indirect DMAs + partial reduce into
             a per-tile accumulator. Keeps the static instruction count
             ~100/layer while the ~3300 dynamic gathers run ~1.45us each.
  AllGather: g2 -> tab2
  layer 2  : same gather, then W2 matmul + log_softmax (For_i) -> out

Host prep: graph partition by dst across cores, degree-sorted ELL layout,
dinv scales, per-rectangle index tables (natural order for layer 1,
degree-sorted order for layer 2). Output rows un-permuted on host.
"""
import sys
sys.path.insert(0, "/opt/trn_rl_repo")
import numpy as np
import ml_dtypes

import concourse.bass as bass
from concourse.bass import ds, ts
import concourse.bacc as bacc
import concourse.mybir as mybir
import concourse.tile as tile
import concourse.bass_utils as bass_utils
from concourse.masks import make_identity

F32 = mybir.dt.float32
BF16 = mybir.dt.bfloat16
FP8 = mybir.dt.float8e4
I32 = mybir.dt.int32

M_CORES = 8


def _rectangles(KS, max_rects=8):
    """Cover the (descending) ELL column staircase with rectangles.

    Returns [(n_tiles, c_lo, c_hi)]: rectangle = tiles [0, n_tiles) x
    columns [c_lo, c_hi). Greedy: split at the largest staircase drops.
    """
    KS = list(KS)
    NT = len(KS)
    assert all(KS[i] >= KS[i + 1] for i in range(NT - 1)), "KS must descend"
    # candidate breakpoints: distinct K values (descending staircase)
    # choose levels greedily by waste reduction
    levels = sorted(set(KS))           # ascending
    base = levels[0]
    chosen = {0, base}
    # waste reduction of adding level c between existing neighbours:
    # evaluate greedily
    def total_waste(lvls):
        lv = sorted(lvls)
        waste = 0
        for t, k in enumerate(KS):
            # covered columns: for each adjacent pair (a, b] need n_tiles with
            # K >= b; per tile, covered = smallest chosen level >= k
            cov = min(l for l in lv if l >= k)
            waste += cov - k
        return waste
    chosen.add(max(KS))
    while len(chosen) < max_rects + 1:
        best, bestw = None, total_waste(chosen)
        for c in levels:
            if c in chosen:
                continue
            w = total_waste(chosen | {c})
            if w < bestw:
                best, bestw = c, w
        if best is None:
            break
        chosen.add(best)
    lv = sorted(c for c in chosen if c > 0)
    rects = []
    prev = 0
    for c in lv:
        n = sum(1 for k in KS if k > prev)       # tiles needing cols > prev
        if n == 0 or c == prev:
            prev = c
            continue
        rects.append((n, prev, c))
        prev = c
    return rects


def _build(NT, D_IN, H, C, KS, rects, n_cores=M_CORES):
    RT = NT * 128
    RTZ = RT + 16                  # 16 trailing zero rows per rank
    KD = D_IN // 128
    NTAB = n_cores * RTZ
    # per-rect index table column offsets
    CR = [n * (hi - lo) for (n, lo, hi) in rects]
    CTOT = int(sum(CR))
    roff = np.concatenate([[0], np.cumsum(CR)]).astype(int)

    nc = bacc.Bacc("TRN2", target_bir_lowering=False, debug=False,
                   num_devices=n_cores)
    xt_ap = nc.dram_tensor("xt", [KD * 128, RT], FP8, kind="ExternalInput").ap()
    w1_ap = nc.dram_tensor("w1", [128, KD * H], BF16, kind="ExternalInput").ap()
    w2_ap = nc.dram_tensor("w2", [H, C], F32, kind="ExternalInput").ap()
    b1_ap = nc.dram_tensor("b1", [128, H], F32, kind="ExternalInput").ap()
    b2_ap = nc.dram_tensor("b2", [128, C], F32, kind="ExternalInput").ap()
    dvn_ap = nc.dram_tensor("dvn", [128, NT], F32, kind="ExternalInput").ap()
    dvp_ap = nc.dram_tensor("dvp", [128, NT], F32, kind="ExternalInput").ap()
    ix1_ap = nc.dram_tensor("ix1", [128, CTOT], I32, kind="ExternalInput").ap()
    ix2_ap = nc.dram_tensor("ix2", [128, CTOT], I32, kind="ExternalInput").ap()
    out_ap = nc.dram_tensor("out", [RT, C], BF16, kind="ExternalOutput").ap()

    rg = [list(range(n_cores))]

    with tile.TileContext(nc) as tc:
        with tc.tile_pool(name="dram", bufs=1, space="DRAM") as dpool, \
             tc.tile_pool(name="const", bufs=1) as cpool, \
             tc.tile_pool(name="work", bufs=4) as wpool, \
             tc.tile_pool(name="gath", bufs=4) as gpool, \
             tc.tile_pool(name="psA", bufs=2, space="PSUM") as psA, \
             tc.tile_pool(name="psT", bufs=2, space="PSUM") as psT:

            # ---- constants
            ident = cpool.tile([128, 128], F32)
            make_identity(nc, ident[:])
            w1_t = cpool.tile([128, KD * H], BF16)
            nc.sync.dma_start(out=w1_t[:], in_=w1_ap[:])
            w2_t = cpool.tile([H, C], F32)
            nc.sync.dma_start(out=w2_t[:], in_=w2_ap[:])
            b1_t = cpool.tile([128, H], F32)
            nc.sync.dma_start(out=b1_t[:], in_=b1_ap[:])
            b2_t = cpool.tile([128, C], F32)
            nc.sync.dma_start(out=b2_t[:], in_=b2_ap[:])
            dvn_t = cpool.tile([128, NT], F32)
            nc.sync.dma_start(out=dvn_t[:], in_=dvn_ap[:])
            dvp_t = cpool.tile([128, NT], F32)
            nc.sync.dma_start(out=dvp_t[:], in_=dvp_ap[:])
            ix1_t = cpool.tile([128, CTOT], I32)
            nc.sync.dma_start(out=ix1_t[:], in_=ix1_ap[:])
            ix2_t = cpool.tile([128, CTOT], I32)
            nc.sync.dma_start(out=ix2_t[:], in_=ix2_ap[:])

            # per-tile partial-sum accumulator [128, NT*H]
            s_acc = cpool.tile([128, NT * H], F32)

            # ---- DRAM intermediates (trailing 16 zero rows per rank feed
            # the ELL padding slots after the AllGather)
            g1l = dpool.tile([RTZ, H], F32)
            tab1 = dpool.tile([NTAB, H], F32, addr_space="Shared")
            g2l = dpool.tile([RTZ, H], F32)
            tab2 = dpool.tile([NTAB, H], F32, addr_space="Shared")

            zt = cpool.tile([128, H], F32)
            nc.vector.memset(zt[:], 0.0)
            nc.sync.dma_start(out=g1l[RT:, :], in_=zt[0:16, :])
            nc.sync.dma_start(out=g2l[RT:, :], in_=zt[0:16, :])

            # ---- phase A: g1 = dvn * (x @ W1); fp8 x tiles cast to bf16
            # during the SWDGE load (HWDGE cannot cast)
            xt_k = xt_ap.rearrange("(k p) c -> p k c", p=128)
            with tc.For_i(0, NT, 1, name="phA") as iv:
                xt_t = gpool.tile([128, KD * 128], BF16, tag="xin")
                nc.gpsimd.dma_start(
                    out=xt_t[:].rearrange("p (k c) -> p k c", k=KD),
                    in_=xt_k[:, :, ds(iv * 128, 128)])
                acc = psA.tile([128, H], F32, tag="accA")
                for k in range(KD):
                    nc.tensor.matmul(
                        out=acc[:],
                        lhsT=xt_t[:, k * 128:(k + 1) * 128],
                        rhs=w1_t[:, k * H:(k + 1) * H],
                        start=(k == 0), stop=(k == KD - 1))
                gt = wpool.tile([128, H], F32, tag="gout")
                nc.vector.tensor_scalar_mul(gt[:], acc[:], dvn_t[:, ts(iv, 1)])
                nc.sync.dma_start(out=g1l[ts(iv, 128), :], in_=gt[:])

            # ---- AllGather 1
            nc.gpsimd.collective_compute(
                "AllGather", mybir.AluOpType.bypass, replica_groups=rg,
                ins=[g1l[:].opt()], outs=[tab1[:].opt()])

            def gather_layer(ix_t, tab):
                """Rectangle loops: gather + partial reduce into s_acc."""
                for r, (n, lo, hi) in enumerate(rects):
                    dc = hi - lo
                    with tc.For_i(0, n, 1, name=f"g{r}") as iv:
                        ixs = gpool.tile([128, dc], I32, tag="ixs")
                        nc.vector.tensor_copy(
                            ixs[:], ix_t[:, ds(int(roff[r]) + iv * dc, dc)])
                        ell = gpool.tile([128, dc * H], F32, tag="ell")
                        for j in range(dc):
                            nc.gpsimd.indirect_dma_start(
                                out=ell[:, j * H:(j + 1) * H],
                                out_offset=None,
                                in_=tab[:],
                                in_offset=bass.IndirectOffsetOnAxis(
                                    ap=ixs[:, j:j + 1], axis=0),
                            )
                        if dc > 1:
                            s = wpool.tile([128, H], F32, tag="s")
                            nc.vector.reduce_sum(
                                out=s[:],
                                in_=ell[:].rearrange("p (k h) -> p h k", h=H),
                                axis=mybir.AxisListType.X)
                        else:
                            s = ell
                        if r == 0:
                            nc.any.tensor_copy(s_acc[:, ts(iv, H)], s[:])
                        else:
                            nc.vector.tensor_add(
                                s_acc[:, ts(iv, H)],
                                s_acc[:, ts(iv, H)], s[:])

            # ---- layer 1
            gather_layer(ix1_t, tab1)
            with tc.For_i(0, NT, 1, name="post1") as iv:
                a = wpool.tile([128, H], F32, tag="p1a")
                nc.vector.tensor_scalar_mul(
                    a[:], s_acc[:, ts(iv, H)], dvp_t[:, ts(iv, 1)])
                nc.vector.tensor_add(a[:], a[:], b1_t[:])
                r1 = wpool.tile([128, H], F32, tag="p1r")
                nc.scalar.activation(r1[:], a[:],
                                     mybir.ActivationFunctionType.Relu)
                nc.vector.tensor_scalar_mul(r1[:], r1[:], dvp_t[:, ts(iv, 1)])
                nc.sync.dma_start(out=g2l[ts(iv, 128), :], in_=r1[:])

            # ---- AllGather 2
            nc.gpsimd.collective_compute(
                "AllGather", mybir.AluOpType.bypass, replica_groups=rg,
                ins=[g2l[:].opt()], outs=[tab2[:].opt()])

            # ---- layer 2
            gather_layer(ix2_t, tab2)
            with tc.For_i(0, NT, 1, name="post2") as iv:
                a = wpool.tile([128, H], F32, tag="p2a")
                nc.vector.tensor_scalar_mul(
                    a[:], s_acc[:, ts(iv, H)], dvp_t[:, ts(iv, 1)])
                ptr = psT.tile([128, 128], F32, tag="ptr2")
                nc.tensor.transpose(out=ptr[:H, :], in_=a[:, :],
                                    identity=ident[:])
                aT = wpool.tile([H, 128], F32, tag="aT")
                nc.any.tensor_copy(aT[:], ptr[:H, :])
                lg = psA.tile([128, C], F32, tag="lg")
                nc.tensor.matmul(out=lg[:], lhsT=aT[:], rhs=w2_t[:],
                                 start=True, stop=True)
                z = wpool.tile([128, C], F32, tag="z")
                nc.vector.tensor_add(z[:], lg[:], b2_t[:])
                mx = wpool.tile([128, 1], F32, tag="mx")
                nc.vector.reduce_max(out=mx[:], in_=z[:],
                                     axis=mybir.AxisListType.X)
                nc.vector.tensor_scalar(
                    out=z[:], in0=z[:], scalar1=mx[:, 0:1], scalar2=None,
                    op0=mybir.AluOpType.subtract)
                e = wpool.tile([128, C], F32, tag="e")
                nc.scalar.activation(e[:], z[:],
                                     mybir.ActivationFunctionType.Exp)
                se = wpool.tile([128, 1], F32, tag="se")
                nc.vector.reduce_sum(out=se[:], in_=e[:],
                                     axis=mybir.AxisListType.X)
                ls = wpool.tile([128, 1], F32, tag="ls")
                nc.scalar.activation(ls[:], se[:],
                                     mybir.ActivationFunctionType.Ln)
                nc.vector.tensor_scalar(
                    out=z[:], in0=z[:], scalar1=ls[:, 0:1], scalar2=None,
                    op0=mybir.AluOpType.subtract)
                nc.gpsimd.dma_start(out=out_ap[ts(iv, 128), :], in_=z[:])

    nc.compile()
    return nc


def _host_prep(x, edge_index, W1, b1, W2, b2, n_cores=M_CORES):
    x = np.asarray(x, dtype=np.float32)
    N, D_IN = x.shape
    W1 = np.asarray(W1, np.float32)
    W2 = np.asarray(W2, np.float32)
    H = W1.shape[1]
    C = W2.shape[1]
    NPC = N // n_cores
    NT = (NPC + 127) // 128
    RT = NT * 128
    RTZ = RT + 16
    ZROW = RT                     # rank 0's zero-row block
    KD = D_IN // 128

    src = np.asarray(edge_index[0], dtype=np.int32)
    dst = np.asarray(edge_index[1], dtype=np.int32)
    deg = np.bincount(dst, minlength=N).astype(np.float64) + 1.0
    dinv = (1.0 / np.sqrt(deg)).astype(np.float32)

    owner = dst // NPC
    np.minimum(owner, n_cores - 1, out=owner)

    per_core = []
    KS_all = np.zeros((n_cores, NT), dtype=np.int64)
    for m in range(n_cores):
        sel = owner == m
        s_m = src[sel]
        d_m = dst[sel] - m * NPC
        s_m = np.concatenate([s_m, np.arange(m * NPC, (m + 1) * NPC,
                                             dtype=np.int32)])
        d_m = np.concatenate([d_m, np.arange(NPC, dtype=np.int32)])
        degl = np.bincount(d_m, minlength=NPC)
        perm = np.argsort(-degl, kind="stable").astype(np.int32)
        inv_perm = np.empty(NPC, dtype=np.int32)
        inv_perm[perm] = np.arange(NPC, dtype=np.int32)
        degs = degl[perm]
        Ks = np.zeros(NT, dtype=np.int64)
        nfull = NPC // 128
        for t in range(nfull):
            Ks[t] = degs[t * 128]
        if NPC % 128:
            Ks[nfull] = degs[nfull * 128] if nfull * 128 < NPC else 0
        per_core.append(dict(s_m=s_m, d_m=d_m, perm=perm, inv_perm=inv_perm,
                             degl=degl))
        KS_all[m] = Ks
    KS = KS_all.max(axis=0)
    KS = np.maximum(KS, 1)

    rects = _rectangles([int(k) for k in KS])
    CR = [n * (hi - lo) for (n, lo, hi) in rects]
    CTOT = int(sum(CR))
    roff = np.concatenate([[0], np.cumsum(CR)]).astype(int)

    # map (tile, col) -> index-table column (per-rectangle layout)
    def ixcol(t_idx, col):
        # returns flat column in the per-rect table; t_idx, col arrays
        res = np.empty_like(t_idx)
        for r, (n, lo, hi) in enumerate(rects):
            selr = (col >= lo) & (col < hi)
            res[selr] = roff[r] + t_idx[selr] * (hi - lo) + (col[selr] - lo)
        return res

    # global node -> table row maps (built once, used per core)
    all_own = np.minimum(np.arange(N) // NPC, n_cores - 1)
    all_loc = np.arange(N) - all_own * NPC
    row_nat = all_own * RTZ + all_loc
    sorted_pos = np.empty(N, dtype=np.int32)
    for j in range(n_cores):
        sorted_pos[j * NPC:(j + 1) * NPC] = per_core[j]["inv_perm"]
    row_prm = all_own * RTZ + sorted_pos

    def table_rows(nodes, permuted):
        return (row_prm if permuted else row_nat)[nodes]

    ix1 = np.full((n_cores, 128, CTOT), ZROW, dtype=np.int32)
    ix2 = np.full((n_cores, 128, CTOT), ZROW, dtype=np.int32)
    dvn = np.ones((n_cores, 128, NT), dtype=np.float32)
    dvp = np.ones((n_cores, 128, NT), dtype=np.float32)

    for m in range(n_cores):
        pc = per_core[m]
        s_m, d_m = pc["s_m"], pc["d_m"]
        spos = pc["inv_perm"][d_m]
        order = np.argsort(spos, kind="stable")
        s_srt = s_m[order]
        p_srt = spos[order]
        counts = pc["degl"][pc["perm"]]
        offs = np.concatenate([[0], np.cumsum(counts)])
        rank = np.arange(len(p_srt)) - offs[p_srt]
        t_idx = p_srt // 128
        p_row = p_srt % 128
        colpos = ixcol(t_idx, rank)
        r1 = table_rows(s_srt, permuted=False)
        r2 = table_rows(s_srt, permuted=True)
        ix1[m, p_row, colpos] = r1
        ix2[m, p_row, colpos] = r2
        own_nodes = np.arange(m * NPC, (m + 1) * NPC)
        dv = dinv[own_nodes]
        nat = np.ones(RT, np.float32)
        nat[:NPC] = dv
        dvn[m] = nat.reshape(NT, 128).T
        prm = np.ones(RT, np.float32)
        prm[:NPC] = dv[pc["perm"]]
        dvp[m] = prm.reshape(NT, 128).T

    x_pad8 = np.zeros((N + RT, D_IN), ml_dtypes.float8_e4m3)
    x_pad8[:N] = x.astype(ml_dtypes.float8_e4m3)
    w1b = np.ascontiguousarray(
        W1.reshape(KD, 128, H).transpose(1, 0, 2).reshape(128, -1)
    ).astype(ml_dtypes.bfloat16)
    b1f = np.tile(np.asarray(b1, np.float32)[None, :], (128, 1))
    b2f = np.tile(np.asarray(b2, np.float32)[None, :], (128, 1))
    in_maps = []
    for m in range(n_cores):
        xs = x_pad8[m * NPC:m * NPC + RT]
        xt = np.ascontiguousarray(xs.T)
        in_maps.append({
            "xt": xt,
            "w1": w1b,
            "w2": W2,
            "b1": b1f,
            "b2": b2f,
            "dvn": dvn[m], "dvp": dvp[m],
            "ix1": ix1[m], "ix2": ix2[m],
        })
    meta = dict(NPC=NPC, NT=NT, RT=RT, KS=[int(k) for k in KS],
                rects=rects, perms=[pc["perm"] for pc in per_core])
    return in_maps, meta


_CACHE = {}


def kernel(x, edge_index, W1, b1, W2, b2):
    x = np.asarray(x)
    n_cores = M_CORES
    N, D_IN = x.shape
    H = np.asarray(W1).shape[1]
    C = np.asarray(W2).shape[1]
    in_maps, meta = _host_prep(x, edge_index, W1, b1, W2, b2, n_cores)
    NPC, NT = meta["NPC"], meta["NT"]
    key = (N, D_IN, H, C, tuple(meta["KS"]))
    if key not in _CACHE:
        _CACHE[key] = _build(NT, D_IN, H, C, meta["KS"], meta["rects"],
                             n_cores)
    nc = _CACHE[key]
    res = bass_utils.run_bass_kernel_spmd(nc, in_maps,
                                          core_ids=list(range(n_cores)))
    out = np.empty((N, C), np.float32)
    for m in range(n_cores):
        om = np.asarray(res.results[m]["out"], dtype=np.float32)
        out[m * NPC + meta["perms"][m]] = om[:NPC]
    return out


# revision 19
# speedup vs baseline: 1.3735x; 1.0392x over previous
"""GCN 2-layer kernel for Trainium2, 8 NeuronCores, single SPMD launch.

out = log_softmax(Ahat @ relu(Ahat @ (x@W1) + b1) @ W2 + b2),
Ahat = D^-1/2 (A+I) D^-1/2.

Rewritten (dinv scaling folded into per-node pre/post scales):
  g1 = dinv * (x @ W1)            [N,16]   bf16 matmul, per-core rows
  s1 = sum_{e: dst=v} g1[src_e]            ELL gather + reduce
  g2 = dinv * relu(dinv * s1 + b1)
  s2 = sum g2[src_e]
  out = log_softmax((dinv * s2) @ W2 + b2)

Single Bass program per core, one launch:
  phase A  : x uploaded fp8 (halves the dominant transfer), cast to bf16
             during the SWDGE load DMA, bf16 matmul W1 -> g1 rows (For_i)
  AllGather: g1 [RT,16] -> tab1 [8*RT,16] (internal shared DRAM)
  layer 1  : ELL gather decomposed into "rectangles" (tile-range x fixed
             column count); each rectangle is one For_i hardware loop whose
             body issues the per-column indirect DMAs + partial reduce into
             a per-tile accumulator. Keeps the static instruction count
             ~100/layer while the ~3300 dynamic gathers run ~1.45us each.
  AllGather: g2 -> tab2
  layer 2  : same gather, then W2 matmul + log_softmax (For_i) -> out

Host prep: graph partition by dst across cores, degree-sorted ELL layout,
dinv scales, per-rectangle index tables (natural order for layer 1,
degree-sorted order for layer 2). Output rows un-permuted on host.
"""
import sys
sys.path.insert(0, "/opt/trn_rl_repo")
import numpy as np
import ml_dtypes

import concourse.bass as bass
from concourse.bass import ds, ts
import concourse.bacc as bacc
import concourse.mybir as mybir
import concourse.tile as tile
import concourse.bass_utils as bass_utils
from concourse.masks import make_identity

F32 = mybir.dt.float32
BF16 = mybir.dt.bfloat16
FP8 = mybir.dt.float8e4
I32 = mybir.dt.int32
I16 = mybir.dt.int16
I8 = mybir.dt.int8

M_CORES = 8


def _rectangles(KS, max_rects=8):
    """Cover the (descending) ELL column staircase with rectangles.

    Returns [(n_tiles, c_lo, c_hi)]: rectangle = tiles [0, n_tiles) x
    columns [c_lo, c_hi). Greedy: split at the largest staircase drops.
    """
    KS = list(KS)
    NT = len(KS)
    assert all(KS[i] >= KS[i + 1] for i in range(NT - 1)), "KS must descend"
    # candidate breakpoints: distinct K values (descending staircase)
    # choose levels greedily by waste reduction
    levels = sorted(set(KS))           # ascending
    base = levels[0]
    chosen = {0, base}
    # waste reduction of adding level c between existing neighbours:
    # evaluate greedily
    def total_waste(lvls):
        lv = sorted(lvls)
        waste = 0
        for t, k in enumerate(KS):
            # covered columns: for each adjacent pair (a, b] need n_tiles with
            # K >= b; per tile, covered = smallest chosen level >= k
            cov = min(l for l in lv if l >= k)
            waste += cov - k
        return waste
    chosen.add(max(KS))
    while len(chosen) < max_rects + 1:
        best, bestw = None, total_waste(chosen)
        for c in levels:
            if c in chosen:
                continue
            w = total_waste(chosen | {c})
            if w < bestw:
                best, bestw = c, w
        if best is None:
            break
        chosen.add(best)
    lv = sorted(c for c in chosen if c > 0)
    rects = []
    prev = 0
    for c in lv:
        n = sum(1 for k in KS if k > prev)       # tiles needing cols > prev
        if n == 0 or c == prev:
            prev = c
            continue
        rects.append((n, prev, c))
        prev = c
    return rects


def _build(NT, D_IN, H, C, KS, rects, n_cores=M_CORES):
    RT = NT * 128
    RTZ = RT + 16                  # 16 trailing zero rows per rank
    KD = D_IN // 128
    NTAB = n_cores * RTZ
    # per-rect index table column offsets
    CR = [n * (hi - lo) for (n, lo, hi) in rects]
    CTOT = int(sum(CR))
    roff = np.concatenate([[0], np.cumsum(CR)]).astype(int)

    nc = bacc.Bacc("TRN2", target_bir_lowering=False, debug=False,
                   num_devices=n_cores)
    xt_ap = nc.dram_tensor("xt", [KD * 128, RT], FP8, kind="ExternalInput").ap()
    w1_ap = nc.dram_tensor("w1", [128, KD * H], BF16, kind="ExternalInput").ap()
    w2_ap = nc.dram_tensor("w2", [H, C], F32, kind="ExternalInput").ap()
    b1_ap = nc.dram_tensor("b1", [128, H], F32, kind="ExternalInput").ap()
    b2_ap = nc.dram_tensor("b2", [128, C], F32, kind="ExternalInput").ap()
    dvn_ap = nc.dram_tensor("dvn", [128, NT], F32, kind="ExternalInput").ap()
    dvp_ap = nc.dram_tensor("dvp", [128, NT], F32, kind="ExternalInput").ap()
    ixl1_ap = nc.dram_tensor("ixl1", [128, CTOT], I16, kind="ExternalInput").ap()
    ixl2_ap = nc.dram_tensor("ixl2", [128, CTOT], I16, kind="ExternalInput").ap()
    ixo_ap = nc.dram_tensor("ixo", [128, CTOT], I8, kind="ExternalInput").ap()
    out_ap = nc.dram_tensor("out", [RT, C], BF16, kind="ExternalOutput").ap()

    rg = [list(range(n_cores))]

    with tile.TileContext(nc) as tc:
        with tc.tile_pool(name="dram", bufs=1, space="DRAM") as dpool, \
             tc.tile_pool(name="const", bufs=1) as cpool, \
             tc.tile_pool(name="work", bufs=4) as wpool, \
             tc.tile_pool(name="gath", bufs=4) as gpool, \
             tc.tile_pool(name="psA", bufs=2, space="PSUM") as psA, \
             tc.tile_pool(name="psT", bufs=2, space="PSUM") as psT:

            # ---- constants
            ident = cpool.tile([128, 128], F32)
            make_identity(nc, ident[:])
            w1_t = cpool.tile([128, KD * H], BF16)
            nc.sync.dma_start(out=w1_t[:], in_=w1_ap[:])
            w2_t = cpool.tile([H, C], F32)
            nc.sync.dma_start(out=w2_t[:], in_=w2_ap[:])
            b1_t = cpool.tile([128, H], F32)
            nc.sync.dma_start(out=b1_t[:], in_=b1_ap[:])
            b2_t = cpool.tile([128, C], F32)
            nc.sync.dma_start(out=b2_t[:], in_=b2_ap[:])
            dvn_t = cpool.tile([128, NT], F32)
            nc.sync.dma_start(out=dvn_t[:], in_=dvn_ap[:])
            dvp_t = cpool.tile([128, NT], F32)
            nc.sync.dma_start(out=dvp_t[:], in_=dvp_ap[:])
            ixl1_t = cpool.tile([128, CTOT], I16)
            nc.sync.dma_start(out=ixl1_t[:], in_=ixl1_ap[:])
            ixl2_t = cpool.tile([128, CTOT], I16)
            nc.sync.dma_start(out=ixl2_t[:], in_=ixl2_ap[:])
            ixo_t = cpool.tile([128, CTOT], I8)
            nc.sync.dma_start(out=ixo_t[:], in_=ixo_ap[:])

            # per-tile partial-sum accumulator [128, NT*H]
            s_acc = cpool.tile([128, NT * H], F32)

            # ---- DRAM intermediates (trailing 16 zero rows per rank feed
            # the ELL padding slots after the AllGather)
            g1l = dpool.tile([RTZ, H], F32)
            tab1 = dpool.tile([NTAB, H], F32, addr_space="Shared")
            g2l = dpool.tile([RTZ, H], F32)
            tab2 = dpool.tile([NTAB, H], F32, addr_space="Shared")

            zt = cpool.tile([128, H], F32)
            nc.vector.memset(zt[:], 0.0)
            nc.sync.dma_start(out=g1l[RT:, :], in_=zt[0:16, :])
            nc.sync.dma_start(out=g2l[RT:, :], in_=zt[0:16, :])

            # ---- phase A: g1 = dvn * (x @ W1); fp8 x tiles cast to bf16
            # during the SWDGE load (HWDGE cannot cast)
            xt_k = xt_ap.rearrange("(k p) c -> p k c", p=128)
            with tc.For_i(0, NT, 1, name="phA") as iv:
                xt_t = gpool.tile([128, KD * 128], BF16, tag="xin")
                nc.gpsimd.dma_start(
                    out=xt_t[:].rearrange("p (k c) -> p k c", k=KD),
                    in_=xt_k[:, :, ds(iv * 128, 128)])
                acc = psA.tile([128, H], F32, tag="accA")
                for k in range(KD):
                    nc.tensor.matmul(
                        out=acc[:],
                        lhsT=xt_t[:, k * 128:(k + 1) * 128],
                        rhs=w1_t[:, k * H:(k + 1) * H],
                        start=(k == 0), stop=(k == KD - 1))
                gt = wpool.tile([128, H], F32, tag="gout")
                nc.vector.tensor_scalar_mul(gt[:], acc[:], dvn_t[:, ts(iv, 1)])
                nc.sync.dma_start(out=g1l[ts(iv, 128), :], in_=gt[:])

            # ---- AllGather 1
            nc.gpsimd.collective_compute(
                "AllGather", mybir.AluOpType.bypass, replica_groups=rg,
                ins=[g1l[:].opt()], outs=[tab1[:].opt()])

            def gather_layer(ixl_t, lname, tab):
                """Rectangle loops: gather + partial reduce into s_acc.

                Table row = own * RTZ + loc is reassembled on-device from the
                int8 owner and int16 local-row uploads (25% less ix traffic).
                """
                for r, (n, lo, hi) in enumerate(rects):
                    dc = hi - lo
                    with tc.For_i(0, n, 1, name=f"{lname}{r}") as iv:
                        ixs = gpool.tile([128, dc], I32, tag="ixs")
                        nc.vector.tensor_copy(
                            ixs[:], ixo_t[:, ds(int(roff[r]) + iv * dc, dc)])
                        nc.vector.tensor_scalar(
                            out=ixs[:], in0=ixs[:], scalar1=RTZ, scalar2=None,
                            op0=mybir.AluOpType.mult)
                        ixt = gpool.tile([128, dc], I32, tag="ixt")
                        nc.vector.tensor_copy(
                            ixt[:], ixl_t[:, ds(int(roff[r]) + iv * dc, dc)])
                        nc.vector.tensor_add(ixs[:], ixs[:], ixt[:])
                        ell = gpool.tile([128, dc * H], F32, tag="ell")
                        for j in range(dc):
                            nc.gpsimd.indirect_dma_start(
                                out=ell[:, j * H:(j + 1) * H],
                                out_offset=None,
                                in_=tab[:],
                                in_offset=bass.IndirectOffsetOnAxis(
                                    ap=ixs[:, j:j + 1], axis=0),
                            )
                        if dc > 1:
                            s = wpool.tile([128, H], F32, tag="s")
                            nc.vector.reduce_sum(
                                out=s[:],
                                in_=ell[:].rearrange("p (k h) -> p h k", h=H),
                                axis=mybir.AxisListType.X)
                        else:
                            s = ell
                        if r == 0:
                            nc.any.tensor_copy(s_acc[:, ts(iv, H)], s[:])
                        else:
                            nc.vector.tensor_add(
                                s_acc[:, ts(iv, H)],
                                s_acc[:, ts(iv, H)], s[:])

            # ---- layer 1
            gather_layer(ixl1_t, "gA", tab1)
            with tc.For_i(0, NT, 1, name="post1") as iv:
                a = wpool.tile([128, H], F32, tag="p1a")
                nc.vector.tensor_scalar_mul(
                    a[:], s_acc[:, ts(iv, H)], dvp_t[:, ts(iv, 1)])
                nc.vector.tensor_add(a[:], a[:], b1_t[:])
                r1 = wpool.tile([128, H], F32, tag="p1r")
                nc.scalar.activation(r1[:], a[:],
                                     mybir.ActivationFunctionType.Relu)
                nc.vector.tensor_scalar_mul(r1[:], r1[:], dvp_t[:, ts(iv, 1)])
                nc.sync.dma_start(out=g2l[ts(iv, 128), :], in_=r1[:])

            # ---- AllGather 2
            nc.gpsimd.collective_compute(
                "AllGather", mybir.AluOpType.bypass, replica_groups=rg,
                ins=[g2l[:].opt()], outs=[tab2[:].opt()])

            # ---- layer 2
            gather_layer(ixl2_t, "gB", tab2)
            with tc.For_i(0, NT, 1, name="post2") as iv:
                a = wpool.tile([128, H], F32, tag="p2a")
                nc.vector.tensor_scalar_mul(
                    a[:], s_acc[:, ts(iv, H)], dvp_t[:, ts(iv, 1)])
                ptr = psT.tile([128, 128], F32, tag="ptr2")
                nc.tensor.transpose(out=ptr[:H, :], in_=a[:, :],
                                    identity=ident[:])
                aT = wpool.tile([H, 128], F32, tag="aT")
                nc.any.tensor_copy(aT[:], ptr[:H, :])
                lg = psA.tile([128, C], F32, tag="lg")
                nc.tensor.matmul(out=lg[:], lhsT=aT[:], rhs=w2_t[:],
                                 start=True, stop=True)
                z = wpool.tile([128, C], F32, tag="z")
                nc.vector.tensor_add(z[:], lg[:], b2_t[:])
                mx = wpool.tile([128, 1], F32, tag="mx")
                nc.vector.reduce_max(out=mx[:], in_=z[:],
                                     axis=mybir.AxisListType.X)
                nc.vector.tensor_scalar(
                    out=z[:], in0=z[:], scalar1=mx[:, 0:1], scalar2=None,
                    op0=mybir.AluOpType.subtract)
                e = wpool.tile([128, C], F32, tag="e")
                nc.scalar.activation(e[:], z[:],
                                     mybir.ActivationFunctionType.Exp)
                se = wpool.tile([128, 1], F32, tag="se")
                nc.vector.reduce_sum(out=se[:], in_=e[:],
                                     axis=mybir.AxisListType.X)
                ls = wpool.tile([128, 1], F32, tag="ls")
                nc.scalar.activation(ls[:], se[:],
                                     mybir.ActivationFunctionType.Ln)
                nc.vector.tensor_scalar(
                    out=z[:], in0=z[:], scalar1=ls[:, 0:1], scalar2=None,
                    op0=mybir.AluOpType.subtract)
                nc.gpsimd.dma_start(out=out_ap[ts(iv, 128), :], in_=z[:])

    nc.compile()
    return nc


def _host_prep(x, edge_index, W1, b1, W2, b2, n_cores=M_CORES):
    x = np.asarray(x, dtype=np.float32)
    N, D_IN = x.shape
    W1 = np.asarray(W1, np.float32)
    W2 = np.asarray(W2, np.float32)
    H = W1.shape[1]
    C = W2.shape[1]
    NPC = N // n_cores
    NT = (NPC + 127) // 128
    RT = NT * 128
    RTZ = RT + 16
    ZROW = RT                     # rank 0's zero-row block
    KD = D_IN // 128

    src = np.asarray(edge_index[0], dtype=np.int32)
    dst = np.asarray(edge_index[1], dtype=np.int32)
    deg = np.bincount(dst, minlength=N).astype(np.float64) + 1.0
    dinv = (1.0 / np.sqrt(deg)).astype(np.float32)

    owner = dst // NPC
    np.minimum(owner, n_cores - 1, out=owner)

    per_core = []
    KS_all = np.zeros((n_cores, NT), dtype=np.int64)
    for m in range(n_cores):
        sel = owner == m
        s_m = src[sel]
        d_m = dst[sel] - m * NPC
        s_m = np.concatenate([s_m, np.arange(m * NPC, (m + 1) * NPC,
                                             dtype=np.int32)])
        d_m = np.concatenate([d_m, np.arange(NPC, dtype=np.int32)])
        degl = np.bincount(d_m, minlength=NPC)
        perm = np.argsort(-degl, kind="stable").astype(np.int32)
        inv_perm = np.empty(NPC, dtype=np.int32)
        inv_perm[perm] = np.arange(NPC, dtype=np.int32)
        degs = degl[perm]
        Ks = np.zeros(NT, dtype=np.int64)
        nfull = NPC // 128
        for t in range(nfull):
            Ks[t] = degs[t * 128]
        if NPC % 128:
            Ks[nfull] = degs[nfull * 128] if nfull * 128 < NPC else 0
        per_core.append(dict(s_m=s_m, d_m=d_m, perm=perm, inv_perm=inv_perm,
                             degl=degl))
        KS_all[m] = Ks
    KS = KS_all.max(axis=0)
    KS = np.maximum(KS, 1)

    rects = _rectangles([int(k) for k in KS])
    CR = [n * (hi - lo) for (n, lo, hi) in rects]
    CTOT = int(sum(CR))
    roff = np.concatenate([[0], np.cumsum(CR)]).astype(int)

    # map (tile, col) -> index-table column (per-rectangle layout)
    def ixcol(t_idx, col):
        # returns flat column in the per-rect table; t_idx, col arrays
        res = np.empty_like(t_idx)
        for r, (n, lo, hi) in enumerate(rects):
            selr = (col >= lo) & (col < hi)
            res[selr] = roff[r] + t_idx[selr] * (hi - lo) + (col[selr] - lo)
        return res

    # global node -> table row maps (built once, used per core)
    all_own = np.minimum(np.arange(N) // NPC, n_cores - 1)
    all_loc = np.arange(N) - all_own * NPC
    own8 = all_own.astype(np.int8)
    all_loc16 = all_loc.astype(np.int16)
    sorted_pos16 = np.empty(N, dtype=np.int16)
    for j in range(n_cores):
        sorted_pos16[j * NPC:(j + 1) * NPC] = per_core[j]["inv_perm"]

    ixl1 = np.full((n_cores, 128, CTOT), ZROW, dtype=np.int16)
    ixl2 = np.full((n_cores, 128, CTOT), ZROW, dtype=np.int16)
    ixo = np.zeros((n_cores, 128, CTOT), dtype=np.int8)
    dvn = np.ones((n_cores, 128, NT), dtype=np.float32)
    dvp = np.ones((n_cores, 128, NT), dtype=np.float32)

    for m in range(n_cores):
        pc = per_core[m]
        s_m, d_m = pc["s_m"], pc["d_m"]
        spos = pc["inv_perm"][d_m]
        order = np.argsort(spos, kind="stable")
        s_srt = s_m[order]
        p_srt = spos[order]
        counts = pc["degl"][pc["perm"]]
        offs = np.concatenate([[0], np.cumsum(counts)])
        rank = np.arange(len(p_srt)) - offs[p_srt]
        t_idx = p_srt // 128
        p_row = p_srt % 128
        colpos = ixcol(t_idx, rank)
        ixl1[m, p_row, colpos] = all_loc16[s_srt]
        ixl2[m, p_row, colpos] = sorted_pos16[s_srt]
        ixo[m, p_row, colpos] = own8[s_srt]
        own_nodes = np.arange(m * NPC, (m + 1) * NPC)
        dv = dinv[own_nodes]
        nat = np.ones(RT, np.float32)
        nat[:NPC] = dv
        dvn[m] = nat.reshape(NT, 128).T
        prm = np.ones(RT, np.float32)
        prm[:NPC] = dv[pc["perm"]]
        dvp[m] = prm.reshape(NT, 128).T

    x_pad8 = np.zeros((N + RT, D_IN), ml_dtypes.float8_e4m3)
    x_pad8[:N] = x.astype(ml_dtypes.float8_e4m3)
    w1b = np.ascontiguousarray(
        W1.reshape(KD, 128, H).transpose(1, 0, 2).reshape(128, -1)
    ).astype(ml_dtypes.bfloat16)
    b1f = np.tile(np.asarray(b1, np.float32)[None, :], (128, 1))
    b2f = np.tile(np.asarray(b2, np.float32)[None, :], (128, 1))
    in_maps = []
    for m in range(n_cores):
        xs = x_pad8[m * NPC:m * NPC + RT]
        xt = np.ascontiguousarray(xs.T)
        in_maps.append({
            "xt": xt,
            "w1": w1b,
            "w2": W2,
            "b1": b1f,
            "b2": b2f,
            "dvn": dvn[m], "dvp": dvp[m],
            "ixl1": ixl1[m], "ixl2": ixl2[m], "ixo": ixo[m],
        })
    meta = dict(NPC=NPC, NT=NT, RT=RT, KS=[int(k) for k in KS],
                rects=rects, perms=[pc["perm"] for pc in per_core])
    return in_maps, meta


_CACHE = {}


def kernel(x, edge_index, W1, b1, W2, b2):
    x = np.asarray(x)
    n_cores = M_CORES
    N, D_IN = x.shape
    H = np.asarray(W1).shape[1]
    C = np.asarray(W2).shape[1]
    in_maps, meta = _host_prep(x, edge_index, W1, b1, W2, b2, n_cores)
    NPC, NT = meta["NPC"], meta["NT"]
    key = (N, D_IN, H, C, tuple(meta["KS"]))
    if key not in _CACHE:
        _CACHE[key] = _build(NT, D_IN, H, C, meta["KS"], meta["rects"],
                             n_cores)
    nc = _CACHE[key]
    res = bass_utils.run_bass_kernel_spmd(nc, in_maps,
                                          core_ids=list(range(n_cores)))
    out = np.empty((N, C), np.float32)
    for m in range(n_cores):
        om = np.asarray(res.results[m]["out"], dtype=np.float32)
        out[m * NPC + meta["perms"][m]] = om[:NPC]
    return out
